# revision 1
# baseline (speedup 1.0000x reference)
"""Guided filter (r=40, eps=1e-3) on 8 Trainium2 NeuronCores.

Sharding: pure data-parallel over the batch dim (8 batches -> 8 cores).
Each core processes 3 channel-images of 512x512.

Algorithm per image:
  box2d(x) done as two banded matmuls on the TensorEngine (version "A":
  the image chunk is the stationary operand, the 0/1 band matrix the
  moving operand; contraction runs over the partition dim so each pass
  both box-filters one axis and transposes the layout).
  - V-pass band columns carry 2^round(log2(1/n_h)) (exact in bf16); the
    per-row residual rho_h is applied later as a per-partition scalar.
  - W-pass band columns carry bf16(1/n_w).
  - eps is added to the II box output via a rank-1 (K=1) matmul.
  Elementwise stage on VectorE/ScalarE consuming PSUM directly.
"""

import os
import sys
import numpy as np
import ml_dtypes
from contextlib import ExitStack

sys.path.insert(0, "/opt/trn_rl_repo")

import concourse.bass as bass
import concourse.tile as tile
from concourse import bacc, mybir
from concourse.bass_utils import run_bass_kernel_spmd

F32 = mybir.dt.float32
BF16 = mybir.dt.bfloat16
ALU = mybir.AluOpType

R = 40
EPS = 1e-3
HW_ = 512
NB = 4  # 128-row blocks per axis
CH = 3  # channels per batch
P = 128
NCORES = 8



_MUL_RECIP_OP = None


def _get_mul_recip_op():
    """Register a fused custom-DVE op: out = Src1 * recip_approx(Src0),
    BITWISE_NOT exponent-flip seed + one inline Newton step (~0.4% rel err,
    one DVE pass instead of reciprocal_approx_fast + tensor_mul)."""
    global _MUL_RECIP_OP
    if _MUL_RECIP_OP is not None:
        return _MUL_RECIP_OP
    import re
    import concourse.dve_ops as dops
    from concourse.dve_spec import AluOp, Bin, C0, C1, Spec, Src0, Src1

    name = "MUL_RECIP_EPS_GF"
    from concourse.dve_spec import C2
    _x = Src0 + C2
    _not_x = Bin(AluOp.BITWISE_NOT, _x, _x)
    _y0 = _not_x * C0

    def _ref(in0, in1, c0, c1, c2):
        x = in0 + c2
        not_x = (~x.view(np.int32)).view(np.float32)
        y0 = not_x * c0
        return in1 * (y0 * (c1 - x * y0))

    op = dops.DveOp(
        name, Spec(body=Src1 * (_y0 * (C1 - _x * _y0)), reference=_ref),
        subdim=False, uops_sha={})
    dops.OPS.append(op)
    dops.CUSTOM_DVE_SPECS[name] = op.spec
    dops._SUB_OPCODE_FOR_NAME[name] = max(dops._SUB_OPCODE_FOR_NAME.values()) + 1
    for ver in ("v3", "v4"):
        try:
            op.compile(ver)
        except ValueError as e:
            m = re.search(r'uops_sha\["%s"\]="([0-9a-f]+)"' % ver, str(e))
            if not m:
                raise
            op.uops_sha[ver] = m.group(1)
            dops._COMPILE_CACHE.pop((name, ver), None)
            op.compile(ver)
    _MUL_RECIP_OP = op
    return op


_SQSUB_OP = None


def _get_sqsub_op():
    """Fused custom-DVE op: out = Src0*C0 - Src1*Src1 (mean_II*rho - mean_I^2)."""
    global _SQSUB_OP
    if _SQSUB_OP is not None:
        return _SQSUB_OP
    import re
    import concourse.dve_ops as dops
    from concourse.dve_spec import C0, Spec, Src0, Src1

    name = "SQSUB_GF"

    def _ref(in0, in1, c0, c1, c2):
        return in0 * c0 - in1 * in1

    op = dops.DveOp(
        name, Spec(body=Src0 * C0 - Src1 * Src1, reference=_ref),
        subdim=False, uops_sha={})
    dops.OPS.append(op)
    dops.CUSTOM_DVE_SPECS[name] = op.spec
    dops._SUB_OPCODE_FOR_NAME[name] = max(dops._SUB_OPCODE_FOR_NAME.values()) + 1
    for ver in ("v3", "v4"):
        try:
            op.compile(ver)
        except ValueError as e:
            m = re.search(r'uops_sha\["%s"\]="([0-9a-f]+)"' % ver, str(e))
            if not m:
                raise
            op.uops_sha[ver] = m.group(1)
            dops._COMPILE_CACHE.pop((name, ver), None)
            op.compile(ver)
    _SQSUB_OP = op
    return op


def _band_range(c):
    n0 = max(0, P * c - R)
    n1 = min(HW_, P * c + P + R)
    return n0, n1


def make_consts():
    idx = np.arange(HW_)
    n1d = (np.minimum(idx + R, HW_ - 1) - np.maximum(idx - R, 0) + 1).astype(np.float64)
    inv_n = 1.0 / n1d
    E = np.round(np.log2(inv_n))
    po2 = 2.0 ** E                      # exact in bf16
    rho = (inv_n * 2.0 ** (-E)).astype(np.float32)   # residual, ~[0.7, 1.42]

    mask = (np.abs(idx[:, None] - idx[None, :]) <= R)
    bandV = (mask * po2[None, :]).astype(ml_dtypes.bfloat16)
    bandW = (mask * inv_n[None, :]).astype(ml_dtypes.bfloat16)
    # [512k, 512n] -> [128 kp, 4*512 (kb, n)]
    bandV = np.ascontiguousarray(
        bandV.reshape(NB, P, HW_).transpose(1, 0, 2).reshape(P, NB * HW_))
    bandW = np.ascontiguousarray(
        bandW.reshape(NB, P, HW_).transpose(1, 0, 2).reshape(P, NB * HW_))

    rho_t = np.ascontiguousarray(rho.reshape(NB, P).T)          # [128, 4]
    return {"bandV": bandV, "bandW": bandW, "rho": rho_t}


def _img_view(dram_ap, c):
    # [3, 512, 512] DRAM tensor -> channel c as [128 hp, 4 hb, 512 w]
    return dram_ap[c].rearrange("(hb hp) w -> hp hb w", hp=P)


def _sb3(t):
    # [128, 2048] SBUF tile AP -> [128, 4, 512]
    return t[:].rearrange("p (hb w) -> p hb w", w=HW_)


def build_model():
    nc = bacc.Bacc("TRN2", target_bir_lowering=False, debug=False,
                   num_devices=NCORES)
    I_d = nc.dram_tensor("I", [CH, HW_, HW_], F32, kind="ExternalInput").ap()
    p_d = nc.dram_tensor("p", [CH, HW_, HW_], F32, kind="ExternalInput").ap()
    bandV_d = nc.dram_tensor("bandV", [P, NB * HW_], BF16, kind="ExternalInput").ap()
    bandW_d = nc.dram_tensor("bandW", [P, NB * HW_], BF16, kind="ExternalInput").ap()
    rho_d = nc.dram_tensor("rho", [P, NB], F32, kind="ExternalInput").ap()
    out_d = nc.dram_tensor("out", [CH, HW_, HW_], F32, kind="ExternalOutput").ap()

    with tile.TileContext(nc) as tc:
        with ExitStack() as ctx:
            build_kernel(ctx, tc, I_d, p_d, out_d,
                         bandV_d, bandW_d, rho_d)
    nc.compile()
    return nc


def build_kernel(ctx, tc, I_d, p_d, out_d, bandV_d, bandW_d, rho_d):
    nc = tc.nc
    FW = NB * HW_  # 2048

    # start the first image's input DMAs before the (large) band consts
    pIf = ctx.enter_context(tc.tile_pool(name="If", bufs=2))
    pPf = ctx.enter_context(tc.tile_pool(name="Pf", bufs=2))
    If0 = pIf.tile([P, FW], F32, tag="If", name="If")
    pf0 = pPf.tile([P, FW], F32, tag="pf", name="pf")
    nc.sync.dma_start(_sb3(If0), _img_view(I_d, 0))
    nc.sync.dma_start(_sb3(pf0), _img_view(p_d, 0))

    consts = ctx.enter_context(tc.tile_pool(name="consts", bufs=1))
    bandV = consts.tile_from(bandV_d)
    bandW = consts.tile_from(bandW_d)
    rho = consts.tile_from(rho_d)

    pBf = ctx.enter_context(tc.tile_pool(name="ibf", bufs=2))
    pY = ctx.enter_context(tc.tile_pool(name="ymid", bufs=2))
    pAB = ctx.enter_context(tc.tile_pool(name="ab", bufs=3))
    pOut = ctx.enter_context(tc.tile_pool(name="outp", bufs=2))
    pT = ctx.enter_context(tc.tile_pool(name="tmps", bufs=3))
    pV = ctx.enter_context(tc.tile_pool(name="psv", bufs=2, space="PSUM"))
    pQ = ctx.enter_context(tc.tile_pool(name="psq", bufs=1, space="PSUM"))
    pRR = ctx.enter_context(tc.tile_pool(name="psr", bufs=1, space="PSUM"))

    def vpass(src_bf, band, psum_pool, dst_bf, copy_eng="act"):
        """One banded pass: src [h|w] bf16 -> dst [w|h] bf16 (box over
        partition axis + transpose). 16 MMs + 4 PSUM->SBUF copies."""
        for i in range(NB):
            ps = psum_pool.tile([P, HW_], F32, tag="ps")
            for j in range(NB):
                n0, n1 = _band_range(j)
                nc.tensor.matmul(
                    ps[:, n0:n1],
                    lhsT=src_bf[:, j * HW_ + i * P: j * HW_ + i * P + P],
                    rhs=band[:, j * HW_ + n0: j * HW_ + n1],
                    start=(j == 0), stop=(j == NB - 1))
            eng = copy_eng if copy_eng != "mix" else ("act" if i % 2 else "dve")
            if eng == "dve":
                nc.vector.tensor_copy(dst_bf[:, i * HW_:(i + 1) * HW_], ps[:])
            else:
                nc.scalar.copy(dst_bf[:, i * HW_:(i + 1) * HW_], ps[:])

    def wpass_mm(src_bf, band, q_tile, j):
        """W-direction banded MMs for output h-chunk j into q_tile."""
        for i in range(NB):
            m0, m1 = _band_range(i)
            nc.tensor.matmul(
                q_tile[:, m0:m1],
                lhsT=src_bf[:, i * HW_ + j * P: i * HW_ + j * P + P],
                rhs=band[:, i * HW_ + m0: i * HW_ + m1],
                start=(i == 0), stop=(i == NB - 1))

    def stageA(c):
        """DMA + converts + stage-1 V-passes for image c."""
        if c == 0:
            I_f, p_f = If0, pf0
        else:
            I_f = pIf.tile([P, FW], F32, tag="If", name="If")
            p_f = pPf.tile([P, FW], F32, tag="pf", name="pf")
            nc.sync.dma_start(_sb3(I_f), _img_view(I_d, c))
            nc.sync.dma_start(_sb3(p_f), _img_view(p_d, c))

        I_bf = pBf.tile([P, FW], BF16, tag="Ibf", name="Ibf")
        p_bf = pBf.tile([P, FW], BF16, tag="pbf", name="pbf")
        Ip_bf = pBf.tile([P, FW], BF16, tag="Ipbf", name="Ipbf")
        II_bf = pBf.tile([P, FW], BF16, tag="IIbf", name="IIbf")
        nc.scalar.copy(I_bf[:], I_f[:])
        nc.scalar.copy(p_bf[:], p_f[:])
        nc.gpsimd.tensor_tensor(Ip_bf[:], I_bf[:], p_bf[:], op=ALU.mult)
        nc.scalar.square(II_bf[:], I_f[:])

        yI = pY.tile([P, FW], BF16, tag="yI", name="yI")
        yp = pY.tile([P, FW], BF16, tag="yp", name="yp")
        yIp = pY.tile([P, FW], BF16, tag="yIp", name="yIp")
        yII = pY.tile([P, FW], BF16, tag="yII", name="yII")
        vpass(I_bf, bandV, pV, yI, "act")
        vpass(p_bf, bandV, pV, yp, "act")
        vpass(Ip_bf, bandV, pV, yIp, "dve")
        vpass(II_bf, bandV, pV, yII, "act")
        return dict(I_f=I_f, I_bf=I_bf, yI=yI, yp=yp, yIp=yIp, yII=yII)

    def stageB(st):
        """Stage-2 W-passes + elementwise -> a, b for image state st."""
        yI, yp, yIp, yII = st["yI"], st["yp"], st["yIp"], st["yII"]
        a_bf = pAB.tile([P, FW], BF16, tag="abf", name="abf")
        b_bf = pAB.tile([P, FW], BF16, tag="bbf", name="bbf")
        st["a_bf"], st["b_bf"] = a_bf, b_bf
        for j in range(NB):
            qI = pQ.tile([P, HW_], F32, tag="qI")
            qp = pQ.tile([P, HW_], F32, tag="qp")
            qIp = pQ.tile([P, HW_], F32, tag="qIp")
            qII = pQ.tile([P, HW_], F32, tag="qII")
            wpass_mm(yI, bandW, qI, j)
            wpass_mm(yp, bandW, qp, j)
            wpass_mm(yIp, bandW, qIp, j)
            wpass_mm(yII, bandW, qII, j)

            s = rho[:, j:j + 1]
            sl = slice(j * HW_, (j + 1) * HW_)
            mIt = pT.tile([P, HW_], F32, tag="mIt")
            mpt = pT.tile([P, HW_], F32, tag="mpt")
            mIj = mIt[:]
            mpj = mpt[:]
            nc.scalar.mul(mIj, qI[:], s)          # mean_I (frees qI)
            nc.scalar.mul(mpj, qp[:], s)          # mean_p (frees qp)
            u = pT.tile([P, HW_], F32, tag="u")
            cov = pT.tile([P, HW_], F32, tag="cov")
            v = pT.tile([P, HW_], F32, tag="v")
            den = pT.tile([P, HW_], F32, tag="den")
            tt = pT.tile([P, HW_], BF16, tag="tt")
            nc.scalar.square(v[:], mIj)
            nc.vector.scalar_tensor_tensor(
                den[:], qII[:], s, v[:], op0=ALU.mult, op1=ALU.subtract)
            nc.gpsimd.tensor_tensor(u[:], mIj, mpj, op=ALU.mult)
            nc.vector.scalar_tensor_tensor(
                cov[:], qIp[:], s, u[:], op0=ALU.mult, op1=ALU.subtract)
            nc.vector._custom_dve(
                _get_mul_recip_op(), out=a_bf[:, sl], in0=den[:], in1=cov[:],
                s0=-0.23549792, s1=2.0017324, imm2=EPS)
            nc.vector.tensor_mul(tt[:], a_bf[:, sl], mIj)
            nc.vector.tensor_sub(b_bf[:, sl], mpj, tt[:])

    def stageC(c, st):
        """Stage-3 box2d(a), box2d(b) + combine + output DMA."""
        a_bf, b_bf, I_f = st["a_bf"], st["b_bf"], st["I_f"]
        ya = pY.tile([P, FW], BF16, tag="ya", name="ya")
        yb = pY.tile([P, FW], BF16, tag="yb", name="yb")
        vpass(a_bf, bandV, pV, ya, "act")
        vpass(b_bf, bandV, pV, yb, "act")

        out_t = pOut.tile([P, FW], F32, tag="out", name="out")
        for j in range(NB):
            ra = pRR.tile([P, HW_], F32, tag="ra", name="ra")
            rb = pRR.tile([P, HW_], F32, tag="rb", name="rb")
            wpass_mm(ya, bandW, ra, j)
            wpass_mm(yb, bandW, rb, j)
            s = rho[:, j:j + 1]
            sl = slice(j * HW_, (j + 1) * HW_)
            f1 = pT.tile([P, HW_], F32, tag="f1", name="f1")
            nc.vector.scalar_tensor_tensor(
                f1[:], ra[:], s, I_f[:, sl], op0=ALU.mult, op1=ALU.mult)
            nc.vector.scalar_tensor_tensor(
                out_t[:, sl], rb[:], s, f1[:], op0=ALU.mult, op1=ALU.add)
            nc.sync.dma_start(_img_view(out_d, c)[:, j, :], out_t[:, sl])

    # software pipeline: PE always has independent V-pass work queued
    # while the previous image's DVE elementwise chain drains.
    st0 = stageA(0)
    stageB(st0)
    st1 = stageA(1)
    stageC(0, st0)
    stageB(st1)
    st2 = stageA(2)
    stageC(1, st1)
    stageB(st2)
    stageC(2, st2)


_NC_CACHE = None
LAST_RESULT = None


def _get_model():
    global _NC_CACHE
    if _NC_CACHE is None:
        _NC_CACHE = build_model()
    return _NC_CACHE


def kernel(I, p):
    global LAST_RESULT
    I = np.asarray(I, dtype=np.float32)
    p = np.asarray(p, dtype=np.float32)
    B = I.shape[0]
    assert I.shape == (B, CH, HW_, HW_), I.shape
    nc = _get_model()
    consts = make_consts()
    in_maps = []
    for k in range(NCORES):
        m = {"I": np.ascontiguousarray(I[k]), "p": np.ascontiguousarray(p[k])}
        m.update(consts)
        in_maps.append(m)
    kwargs = {}
    if os.environ.get("BASS_TRACE_DIR"):
        kwargs["tmpdir"] = os.environ["BASS_TRACE_DIR"]
    res = run_bass_kernel_spmd(nc, in_maps, core_ids=list(range(NCORES)), **kwargs)
    LAST_RESULT = res
    out = np.stack([res.results[k]["out"] for k in range(NCORES)], axis=0)
    return out.astype(np.float32)


if __name__ == "__main__":
    rng = np.random.default_rng(0)
    I = rng.random((8, CH, HW_, HW_), dtype=np.float32)
    p = rng.random((8, CH, HW_, HW_), dtype=np.float32)
    out = kernel(I, p)
    print("out", out.shape, out.dtype, float(out.mean()))

